# revision 36
# baseline (speedup 1.0000x reference)
"""Trainium2 Bass kernel for MFA (mixture of factor analyzers) log-prob.

Data-parallel over N across 8 NeuronCores. Host folds the Woodbury/Cholesky
algebra into small weight matrices; per 128-sample tile the device computes

    ps3[n, k] = (off-B) + w2^T x + w3^T xsq + ind^T usq
                (flipped matmuls: x/xsq/usq tiles are the stationary
                operand, so the result lands in PSUM sample-major with no
                transposes; off-B is seeded by a ones^T @ offk matmul)
    ssum[n]   = sum_k exp(ps3[n, k] - C)

and the host finishes  y = S + log(ssum), S = B + C a global shift chosen
from a small sample of x so every fp32 exp argument stays in-range.

x is pre-transposed/cast to f16 on the host so DMA delivers feature-major
tiles [128 features, n samples]; output ssum is [128, cols] f32.
"""

import math
from concurrent.futures import ThreadPoolExecutor
from contextlib import ExitStack

import numpy as np

import concourse.bass as bass
import concourse.bacc as bacc
import concourse.mybir as mybir
import concourse.tile as tile
from concourse.bass_utils import run_bass_kernel_spmd

N_TOTAL = 500000
D = 128
K = 32
L = 4
N_CORES = 8
N_PER_CORE = N_TOTAL // N_CORES           # 62500
MACRO = 512                               # samples per macro-tile
SUPER = 4                                 # macro-tiles per exp/sum batch
_nm = (N_PER_CORE + MACRO - 1) // MACRO
N_MACROS = ((_nm + SUPER - 1) // SUPER) * SUPER   # 124
N_PAD = N_MACROS * MACRO                  # 63488
N_SUPERS = N_MACROS // SUPER              # 31
N_COLS = N_PAD // 128                     # 496

FP32 = mybir.dt.float32
FP16 = mybir.dt.float16


def _factorize(MU, A, D_, PI):
    Kk, d, l = A.shape
    MU = MU.astype(np.float64)
    A = A.astype(np.float64)
    D_ = D_.astype(np.float64)
    PI = PI.astype(np.float64)

    iD = D_ ** -2.0
    B = iD[:, :, None] * A
    Lm = np.eye(l)[None] + np.einsum('kdl,kdm->klm', A, B)
    iL = np.linalg.inv(Lm)
    C = np.linalg.cholesky(iL)
    W0 = np.einsum('kdl,klm->kdm', B, C)              # [K,d,l]
    c = np.einsum('kd,kdl->kl', MU, W0)

    w3 = -0.5 * iD.T                                  # [d,K]
    w2 = (iD * MU).T - np.einsum('kl,kdl->dk', c, W0)
    Wc = (W0 * math.sqrt(0.5)).transpose(1, 0, 2).reshape(d, Kk * l)
    logdet = np.log(np.linalg.det(Lm)) + np.sum(np.log(D_ ** 2), axis=1)
    t_const = np.sum(iD * MU * MU, axis=1)
    off = PI - 0.5 * (d * math.log(2 * math.pi) + logdet + t_const) \
        + 0.5 * np.sum(c * c, axis=1)
    return Wc, w2, w3, off


def _build_bass():
    nc = bacc.Bacc(None, target_bir_lowering=False)

    PAIR = 2 * MACRO                      # 1024-wide elementwise ops
    GRP = SUPER * 4                       # 16 sample-groups of 128 per super
    WB_COLS = D + 3 * K + D + K           # Wc|w2|w3|ind|ones|offk

    xT = nc.dram_tensor("xT", [D, N_PAD], FP16, kind="ExternalInput")
    wb_d = nc.dram_tensor("wb", [D, WB_COLS], FP16, kind="ExternalInput")
    negc_d = nc.dram_tensor("negc", [128, 1], FP32, kind="ExternalInput")
    y_d = nc.dram_tensor("y", [128, N_COLS], FP32, kind="ExternalOutput")

    with tile.TileContext(nc) as tc, ExitStack() as ctx:
        consts = ctx.enter_context(tc.tile_pool(name="consts", bufs=1))
        xpool = ctx.enter_context(tc.tile_pool(name="xpool", bufs=3))
        sqpool = ctx.enter_context(tc.tile_pool(name="sqpool", bufs=4))
        usqpool = ctx.enter_context(tc.tile_pool(name="usqpool", bufs=4))
        eepool = ctx.enter_context(tc.tile_pool(name="eepool", bufs=4))
        respool = ctx.enter_context(tc.tile_pool(name="respool", bufs=1))
        psU = ctx.enter_context(tc.tile_pool(name="psU", bufs=2, space="PSUM"))
        psC = ctx.enter_context(tc.tile_pool(name="psC", bufs=3, space="PSUM"))

        sb_wb = consts.tile([D, WB_COLS], FP16)
        sb_negc = consts.tile([128, 1], FP32)
        nc.sync.dma_start(out=sb_wb, in_=wb_d[:, :])
        nc.sync.dma_start(out=sb_negc, in_=negc_d[:, :])
        sb_wc = sb_wb[:, 0:D]
        sb_w2 = sb_wb[:, D:D + K]
        sb_w3 = sb_wb[:, D + K:D + 2 * K]
        sb_ind = sb_wb[:, D + 2 * K:D + 3 * K]
        sb_ones = sb_wb[:, D + 3 * K:2 * D + 3 * K]
        sb_offr = sb_wb[:, 2 * D + 3 * K:]

        resbuf = respool.tile([128, N_COLS], FP32)

        for s in range(N_SUPERS):
            sb_xs = xpool.tile([D, SUPER * MACRO], FP16, tag="x")
            nc.sync.dma_start(
                out=sb_xs,
                in_=xT[:, s * SUPER * MACRO:(s + 1) * SUPER * MACRO])

            ps3 = psC.tile([128, GRP, K], FP32, tag="ps3")
            ee = eepool.tile([128, SUPER * 4, K], FP32, tag="ee")
            for j in range(SUPER // 2):          # two 1024-wide pairs
                p = s * 2 + j
                xp = sb_xs[:, j * PAIR:(j + 1) * PAIR]

                sb_xsq = sqpool.tile([D, PAIR], FP16, tag="xsq")
                if p % 9 < 4:
                    nc.gpsimd.tensor_mul(sb_xsq, xp, xp)
                else:
                    nc.vector.tensor_mul(sb_xsq, xp, xp)

                ps_u = psU.tile([K * L, PAIR], FP32, tag="u")
                nc.tensor.matmul(ps_u[:, 0:MACRO], sb_wc, xp[:, 0:MACRO],
                                 start=True, stop=True)
                nc.tensor.matmul(ps_u[:, MACRO:PAIR], sb_wc,
                                 xp[:, MACRO:PAIR], start=True, stop=True)

                sb_usq = usqpool.tile([K * L, PAIR], FP16, tag="usq")
                if p % 9 >= 5:
                    # DVE path: one PSUM read max per op — copy out, square
                    sb_uc = sqpool.tile([K * L, PAIR], FP16, tag="ucp")
                    nc.vector.tensor_copy(sb_uc, ps_u)
                    nc.vector.tensor_mul(sb_usq, sb_uc, sb_uc)
                else:
                    nc.scalar.activation(
                        out=sb_usq, in_=ps_u,
                        func=mybir.ActivationFunctionType.Square)

                for t in range(PAIR // 128):
                    g = j * 8 + t
                    sl = slice(t * 128, (t + 1) * 128)
                    nc.tensor.matmul(
                        ps3[:, g, :], sb_ones, sb_offr,
                        start=True, stop=False)
                    nc.tensor.matmul(
                        ps3[:, g, :], xp[:, sl], sb_w2,
                        start=False, stop=False)
                    nc.tensor.matmul(
                        ps3[:, g, :], sb_xsq[:, sl], sb_w3,
                        start=False, stop=False)
                    nc.tensor.matmul(
                        ps3[:, g, :], sb_usq[:, sl], sb_ind,
                        start=False, stop=True)

            ee = eepool.tile([128, GRP, K], FP32, tag="ee")
            nc.scalar.activation(
                out=ee, in_=ps3, func=mybir.ActivationFunctionType.Exp,
                bias=sb_negc, scale=1.0)
            nc.vector.reduce_sum(
                resbuf[:, s * GRP:(s + 1) * GRP], ee,
                axis=mybir.AxisListType.X)

        nc.sync.dma_start(out=y_d[:, :], in_=resbuf)

    nc.compile()
    return nc


_CACHE = {}


def prepare_in_maps(x, MU, A, D_, PI):
    Wc, w2, w3, off = _factorize(MU, A, D_, PI)

    # Global shift S so every fp32 exp stays in range: sample a few rows of
    # x to locate the typical per-sample max of comp = ps + off.
    xs = np.asarray(x[::max(1, len(x) // 512)][:512], dtype=np.float64)
    ps_s = (xs * xs) @ w3 + xs @ w2
    uu = xs @ Wc
    ps_s += (uu * uu).reshape(len(xs), K, L).sum(axis=2)
    S = float(np.median((ps_s + off[None, :]).max(axis=1)))
    B = float((off.max() + off.min()) / 2.0)
    negC = B - S                                       # exp bias = -C

    ind = np.zeros((K * L, K), dtype=np.float64)
    for k in range(K):
        ind[k * L:(k + 1) * L, k] = 1.0

    ones = np.ones((D, D), dtype=np.float64)
    offk = np.zeros((D, K), dtype=np.float64)
    offk[0, :] = off - B

    wb = np.concatenate([Wc, w2, w3, ind, ones, offk],
                        axis=1).astype(np.float16)
    negc_t = np.full((128, 1), negC, dtype=np.float32)

    xf = np.ascontiguousarray(x, dtype=np.float32)

    def shard(c):
        xs16 = xf[c * N_PER_CORE:(c + 1) * N_PER_CORE].astype(np.float16)
        xTs = np.zeros((128, N_PAD), dtype=np.float16)
        xTs[:, :N_PER_CORE] = xs16.T
        return {"xT": xTs, "wb": wb, "negc": negc_t}

    with ThreadPoolExecutor(max_workers=N_CORES) as ex:
        in_maps = list(ex.map(shard, range(N_CORES)))
    return in_maps, S


def postprocess_core_output(y_dev, S):
    ssum = y_dev.T.reshape(-1)[:N_PER_CORE].astype(np.float64)
    return (S + np.log(ssum)).astype(np.float32)


def kernel(x, MU, A, D, PI):
    in_maps, S = prepare_in_maps(x, MU, A, D, PI)

    if "nc" not in _CACHE:
        _CACHE["nc"] = _build_bass()
    nc = _CACHE["nc"]

    res = run_bass_kernel_spmd(nc, in_maps, core_ids=list(range(N_CORES)))
    outs = []
    for c in range(N_CORES):
        outs.append(postprocess_core_output(res.results[c]["y"], S))
    return np.concatenate(outs).astype(np.float32)
